# revision 1
# baseline (speedup 1.0000x reference)
"""Multi-LoRA batched low-rank adapter kernel for 8 trn2 NeuronCores.

Problem: x [16, 2048, 4096] f32, adapter_ids [16] int, A [64, 4096, 64],
B [64, 64, 4096].  out[b] = (x[b] @ B[id_b].T) @ A[id_b].T * (1/64).

Sharding: data-parallel over batch (2 samples/core); per-sample
adapters are gathered on host (adapter_ids are host-visible and tiny).

The kernel is DMA-bound (~70 MB/core at ~400 GB/s/NC aggregate), so:
 * single bf16 term everywhere (rel err 4.2e-3 measured vs the 2e-2
   gate): x loads hi-only, out stored bf16 and upcast on host — half
   the bytes of the fp32/hi-lo alternative, and 3x fewer PE matmuls.
 * every DRAM operand is pre-swizzled on host so each DMA descriptor
   is a >=2KB contiguous per-partition run (x blocks [p, k, m], B^T
   [p, k, r], A^T rows, and the output is stored in a device-friendly
   [blk, p, ns, n] layout and unswizzled on host).
 * B^T of the core's TWO samples is packed into one [DIN, 128]
   stationary (cols 0:64 = sample0, 64:128 = sample1): mm1 computes
   both samples' Bx rows in one pass per sample; A^T rows are zero
   padded so each sample's mm2 contracts only its own rows.  Saves
   half the B traffic and keeps K=128 (fast weight load).

Structure: per sample, 4 seq-blocks of 512.  Per block: load x^T
block (4x 1MB quarter-DMAs so mm1 starts after the first MB), 32
accumulating mm1 matmuls into one PSUM bank, one DVE drain to bf16,
then 32 mm2 matmuls (stationary = Bx chunk, moving = A^T) drained
DVE/ACT-alternating into a bf16 staging tile, stored as one 4MB
transfer on gpsimd/SWDGE (last block: 4x 1MB to shorten the tail).
mm2 of block g is emitted after mm1 of block g+1 so the bx drain
latency hides under PE work.  Loads ride the SP HWDGE ring, stores
SWDGE, drains split DVE/ACT, so no engine head-of-line blocks a ring.
"""

import numpy as np
from contextlib import ExitStack

import concourse.tile as tile
from concourse import bacc, mybir, bass_utils

NCORES = 8
BATCH = 16
B_PER = BATCH // NCORES
SEQ = 2048
DIN = 4096
DOUT = 4096
RANK = 64
RPAD = 128
SCALE = np.float32(1.0 / 64.0)

f32 = mybir.dt.float32
bf16 = mybir.dt.bfloat16

P = 128
KI = DIN // P       # 32 contraction tiles for mm1
SB = 512            # seq block
NBLK = SEQ // SB    # 4
NSB = SB // P       # 4 output row-chunks per block
OT = DOUT // 512    # 8
XQ = 4              # x quarter-DMAs per block
KQ = KI // XQ       # 8 k-tiles per quarter

_CACHE = {}


def _build_nc(repeat=1):
    nc = bacc.Bacc("TRN2", target_bir_lowering=False, debug=False)
    # all layouts are per-partition contiguous (partition dim explicit)
    xq_d = nc.dram_tensor("xq", [B_PER, NBLK, XQ, P, KQ, SB], bf16,
                          kind="ExternalInput").ap()
    bh_d = nc.dram_tensor("bh", [XQ, P, KQ, RPAD], bf16,
                          kind="ExternalInput").ap()       # pair-packed
    ah_d = nc.dram_tensor("ah", [B_PER, RPAD, DOUT], bf16,
                          kind="ExternalInput").ap()
    out = nc.dram_tensor("out", [B_PER, NBLK, P, NSB, DOUT], bf16,
                         kind="ExternalOutput").ap()

    with tile.TileContext(nc) as tc, ExitStack() as ctx:
        adp = ctx.enter_context(tc.tile_pool(name="adp", bufs=2))
        bhp_pool = ctx.enter_context(tc.tile_pool(name="bhp", bufs=1))
        xbp = ctx.enter_context(tc.tile_pool(name="xbp", bufs=2))
        bxsp = ctx.enter_context(tc.tile_pool(name="bxsp", bufs=2))
        stg = ctx.enter_context(tc.tile_pool(name="stg", bufs=2))
        bxps = ctx.enter_context(tc.tile_pool(name="bxps", bufs=2, space="PSUM"))
        outp = ctx.enter_context(tc.tile_pool(name="outp", bufs=3, space="PSUM"))

        def load_bh():
            t = bhp_pool.tile([P, KI, RPAD], bf16, name="bht", tag="bht")
            for q in range(XQ):
                nc.sync.dma_start(t[:, q * KQ:(q + 1) * KQ, :], bh_d[q])
            return t

        def load_ah(s):
            t = adp.tile([RPAD, DOUT], bf16, name="ah", tag="ah")
            nc.sync.dma_start(t[:], ah_d[s])
            return t

        def mm1_block(s, blk, bht):
            xt = xbp.tile([P, KI, SB], bf16, name="xt", tag="xt")
            for q in range(XQ):
                nc.sync.dma_start(xt[:, q * KQ:(q + 1) * KQ, :],
                                  xq_d[s, blk, q])
            bx = bxps.tile([P, SB], f32, name="bx", tag="bx")
            for k in range(KI):
                nc.tensor.matmul(bx[:], bht[:, k, :], xt[:, k, :],
                                 start=(k == 0), stop=(k == KI - 1))
            bxh = bxsp.tile([RPAD, SB], bf16, name="bxh", tag="bxh")
            nc.vector.tensor_copy(bxh[:], bx[:])
            return bxh

        def mm2_block(s, blk, ah, bxh, last):
            st = stg.tile([P, NSB, DOUT], bf16, name="st", tag="st")
            for ns in range(NSB):
                for otp in range(OT // 2):
                    ps = outp.tile([P, 1024], f32, name="ps_o", tag="ps_o")
                    for half in range(2):
                        ot = otp * 2 + half
                        ov = slice(ot * 512, (ot + 1) * 512)
                        pv = slice(half * 512, (half + 1) * 512)
                        nc.tensor.matmul(ps[:, pv],
                                         bxh[:, ns * P:(ns + 1) * P],
                                         ah[:, ov], start=True, stop=True)
                    dv = slice(otp * 1024, (otp + 1) * 1024)
                    if otp % 2 == 0:
                        nc.vector.tensor_copy(st[:, ns, dv], ps[:])
                    else:
                        nc.scalar.copy(st[:, ns, dv], ps[:])
                if last:
                    nc.gpsimd.dma_start(out[s, blk, :, ns, :], st[:, ns, :])
            if not last:
                nc.gpsimd.dma_start(out[s, blk], st[:])

        samples = [s for _ in range(repeat) for s in range(B_PER)]
        blocks = [(s, blk) for s in samples for blk in range(NBLK)]

        bht = load_bh()
        ahs = [load_ah(samples[0])] + [None] * (len(samples) - 1)
        prev = None
        for g, (s, blk) in enumerate(blocks):
            spos = g // NBLK
            if blk == 2 and spos + 1 < len(samples):
                ahs[spos + 1] = load_ah(samples[spos + 1])
            bxh = mm1_block(s, blk, bht)
            if prev is not None:
                mm2_block(*prev)
            prev = (s, blk, ahs[spos], bxh, g == len(blocks) - 1)
        mm2_block(*prev)
    nc.compile()
    return nc


def _get_nc(repeat=1):
    key = f"nc{repeat}"
    if key not in _CACHE:
        _CACHE[key] = _build_nc(repeat)
    return _CACHE[key]


def _prep_in_maps(x, adapter_ids, A, B):
    import ml_dtypes
    x = np.asarray(x, dtype=np.float32)
    ids = np.asarray(adapter_ids).astype(np.int64)
    A = np.asarray(A, dtype=np.float32)
    B = np.asarray(B, dtype=np.float32)

    As = A * SCALE
    in_maps = []
    for c in range(NCORES):
        sl = slice(c * B_PER, (c + 1) * B_PER)
        cids = ids[sl]
        xT = x[sl].transpose(0, 2, 1)                       # [2, DIN, SEQ]
        # [B_PER, NBLK, XQ, P, KQ, SB]: d = (q*KQ + kq)*P + p, n = blk*SB + m
        xq = xT.reshape(B_PER, XQ, KQ, P, NBLK, SB).transpose(0, 4, 1, 3, 2, 5)
        xq = np.ascontiguousarray(xq)
        # pair-packed B^T: cols 0:64 sample0, 64:128 sample1
        BT = np.concatenate([B[cids[0]].T, B[cids[1]].T], axis=1)  # [DIN, 128]
        bh = np.ascontiguousarray(
            BT.reshape(XQ, KQ, P, RPAD).transpose(0, 2, 1, 3))
        # A^T with per-sample row offset; other rows zero
        AT = np.zeros((B_PER, RPAD, DOUT), np.float32)
        for i in range(B_PER):
            AT[i, i * RANK:(i + 1) * RANK, :] = As[cids[i]].T
        in_maps.append({
            "xq": xq.astype(ml_dtypes.bfloat16),
            "bh": bh.astype(ml_dtypes.bfloat16),
            "ah": AT.astype(ml_dtypes.bfloat16),
        })
    return in_maps


def kernel(x, adapter_ids, A, B):
    nc = _get_nc()
    in_maps = _prep_in_maps(x, adapter_ids, A, B)
    res = bass_utils.run_bass_kernel_spmd(
        nc, in_maps, core_ids=list(range(NCORES)))
    out = np.empty((BATCH, SEQ, DOUT), dtype=np.float32)
    for c in range(NCORES):
        # [B_PER, NBLK, P, NSB, DOUT] -> [B_PER, NBLK, NSB, P, DOUT] -> seq
        o = res.results[c]["out"].astype(np.float32)
        out[c * B_PER:(c + 1) * B_PER] = o.transpose(0, 1, 3, 2, 4).reshape(
            B_PER, SEQ, DOUT)
    return out

